# revision 11
# baseline (speedup 1.0000x reference)
"""EnvironmentConsistentAttention on 8 trn2 cores.

Sharding: 4 images x 2 directions (vertical/horizontal neighbor pairs) = 8
independent units, one per core. The horizontal direction of image x equals
the vertical direction of x spatially transposed, so a single SPMD program
handles both: given shifted maps A, B [31,32,256] it returns
(yA, yB) = _corr_recon(A, B), each [31,32,256] (emitted channel-major).

Per-core math (Hp=31, Wp=32, C=256, L=992, k=3):
  pa[(p,q,c), l=(h,w)] = A_pad[h+p, w+q, c]          (zero-padded patches)
  z = pa * pb                                        [2304, L]
  R = z.T @ z                                        [L, L] gram
  att[i,j] = inv[i]*inv[j]*R[i,j];  S = softmax(10*att, axis=j)
  yA = conv_transpose(S, pa) -> ya[l',c] = sum_{p,q,j} S[shift(l',p,q), j]*pa[(p,q,c), j]

att is symmetric pre-softmax, so tiles of R computed as [j-part, i-free] are
directly S.T tiles; exp/softmax-denominator (a cross-partition ones-matmul)
and the reconstruction all run in that transposed layout. S.T is stored in a
[33,34]-padded spatial grid over i so the 9 conv-transpose shifts become pure
access-pattern offsets (zero borders give SAME-padding semantics for free).
Patch norms are folded in as row/column scales of R (host precomputes the
tiny [992] inverse-norm vector).
"""

import numpy as np

Hp, Wp, C = 31, 32, 256
L = Hp * Wp            # 992
PH, PW = Hp + 2, Wp + 2  # 33, 34 padded grid
NPAD = PH * PW         # 1122
KK = 9 * C             # 2304
JC = [(128 * c, 128 if c < 7 else 96) for c in range(8)]   # j/l chunks
HALves = [(0, 512, 0, 16), (512, 480, 16, 15)]  # (i0, n, h0, nh) over i/l'
B_IMG, H_IMG, W_IMG = 4, 32, 32

_CACHE = {}


def _build_program():
    import concourse.bass as bass
    import concourse.tile as tile
    from concourse import bacc, mybir

    f32 = mybir.dt.float32
    f32r = mybir.dt.float32r

    def r(ap):
        return ap.bitcast(f32r)

    nc = bacc.Bacc("TRN2", target_bir_lowering=False, debug=False)

    a_pad = nc.dram_tensor("a_pad", [PH, PW, C], f32, kind="ExternalInput")
    b_pad = nc.dram_tensor("b_pad", [PH, PW, C], f32, kind="ExternalInput")
    a_chw = nc.dram_tensor("a_chw", [C, NPAD], f32, kind="ExternalInput")
    b_chw = nc.dram_tensor("b_chw", [C, NPAD], f32, kind="ExternalInput")
    inv_p = nc.dram_tensor("inv_p", [128, 8], f32, kind="ExternalInput")
    inv_f = nc.dram_tensor("inv_f", [1, L], f32, kind="ExternalInput")
    ya_t = nc.dram_tensor("ya_t", [C, L], f32, kind="ExternalOutput")
    yb_t = nc.dram_tensor("yb_t", [C, L], f32, kind="ExternalOutput")

    with tile.TileContext(nc) as tc:
        from contextlib import ExitStack

        with ExitStack() as ctx:
            const = ctx.enter_context(tc.tile_pool(name="const", bufs=1))
            outp = ctx.enter_context(tc.tile_pool(name="outp", bufs=4))
            tpadp = ctx.enter_context(tc.tile_pool(name="tpad", bufs=8))

            # Constants
            sb_inv_p = const.tile([128, 8], f32, tag="invp")
            nc.sync.dma_start(out=sb_inv_p[:], in_=inv_p[:, :])
            sb_inv_b = const.tile([128, L], f32, tag="invb")
            nc.sync.dma_start(
                out=sb_inv_b[:], in_=inv_f.ap().to_broadcast([128, L])
            )
            ones_f = const.tile([128, 128], f32, tag="onesf")
            nc.vector.memset(ones_f[:], 1.0)
            ones_k = const.tile([128, 1], f32r, tag="onesk")
            nc.scalar.copy(ones_k[:], ones_f[:, 0:1])
            ones_m = const.tile([1, 128], f32r, tag="onesm")
            nc.scalar.copy(ones_m[:], ones_f[0:1, :])
            recip_sb = const.tile([1, L], f32r, tag="recip")
            rb_sb = const.tile([128, L], f32, tag="rbcast")

            # S.T tiles in padded-grid layout, zeroed borders
            tpad = []
            for c in range(8):
                t = tpadp.tile([128, NPAD], f32r, tag="tpad")
                tf = t.bitcast(f32).rearrange("j (h w) -> j h w", h=PH, w=PW)
                nc.vector.memset(tf[:, 0:1, :], 0.0)
                nc.vector.memset(tf[:, PH - 1 : PH, :], 0.0)
                nc.vector.memset(tf[:, :, 0:1], 0.0)
                nc.vector.memset(tf[:, :, PW - 1 : PW], 0.0)
                tpad.append(t)

            with ExitStack() as ph1:
                apadp = ph1.enter_context(tc.tile_pool(name="apad", bufs=4))
                zp = ph1.enter_context(tc.tile_pool(name="z", bufs=18))
                psA = ph1.enter_context(
                    tc.tile_pool(name="psA", bufs=4, space="PSUM")
                )
                psD = ph1.enter_context(
                    tc.tile_pool(name="psD", bufs=2, space="PSUM")
                )

                # Load padded inputs channel-major; build z = pa*pb views
                achw, bchw = [], []
                for ch in range(2):
                    ta = apadp.tile([128, NPAD], f32, tag="apad")
                    nc.sync.dma_start(
                        out=ta[:], in_=a_chw[128 * ch : 128 * (ch + 1), :]
                    )
                    achw.append(ta)
                    tb = apadp.tile([128, NPAD], f32, tag="apad")
                    nc.gpsimd.dma_start(
                        out=tb[:], in_=b_chw[128 * ch : 128 * (ch + 1), :]
                    )
                    bchw.append(tb)

                zt = []
                for p in range(3):
                    for q in range(3):
                        for ch in range(2):
                            k = len(zt)
                            zk = zp.tile([128, L], f32r, tag="z")
                            av = achw[ch].rearrange(
                                "c (h w) -> c h w", h=PH, w=PW
                            )[:, p : p + Hp, q : q + Wp]
                            bv = bchw[ch].rearrange(
                                "c (h w) -> c h w", h=PH, w=PW
                            )[:, p : p + Hp, q : q + Wp]
                            eng = nc.gpsimd if k % 3 == 2 else nc.vector
                            eng.tensor_mul(zk[:], av, bv)
                            zt.append(zk)

                # Gram R = z.T@z per (j-chunk, i-half); scale+exp into tpad;
                # accumulate softmax denominators with ones-matmuls.
                dps = [psD.tile([1, n], f32, tag="dps", name=f"dps{hi}") for hi, (_, n, _, _) in enumerate(HALves)]
                for c, (j0, dm) in enumerate(JC):
                    rps = [
                        psA.tile([128, n], f32, tag="rps", name=f"rps{c}_{hi}")
                        for hi, (_, n, _, _) in enumerate(HALves)
                    ]
                    for k in range(18):
                        for hi, (i0, n, _, _) in enumerate(HALves):
                            nc.tensor.matmul(
                                rps[hi][:dm, :],
                                zt[k][:, j0 : j0 + dm],
                                zt[k][:, i0 : i0 + n],
                                start=(k == 0),
                                stop=(k == 17),
                            )
                    t3 = tpad[c].rearrange("j (h w) -> j h w", h=PH, w=PW)
                    for hi, (i0, n, h0, nh) in enumerate(HALves):
                        itv = t3[:dm, 1 + h0 : 1 + h0 + nh, 1 : 1 + Wp]
                        nc.vector.tensor_mul(
                            itv, rps[hi][:dm, :], sb_inv_b[:dm, i0 : i0 + n]
                        )
                        nc.scalar.activation(
                            itv,
                            itv,
                            mybir.ActivationFunctionType.Exp,
                            scale=sb_inv_p[:dm, c : c + 1],
                        )
                        nc.tensor.matmul(
                            dps[hi][:, :],
                            ones_k[:dm, :],
                            t3[:dm, 1 + h0 : 1 + h0 + nh, 1 : 1 + Wp],
                            start=(c == 0),
                            stop=(c == 7),
                        )

                # 1/denom, broadcast across partitions via K=1 matmul
                rtmp = const.tile([1, L], f32, tag="rtmp")
                rtmp2 = const.tile([1, L], f32, tag="rtmp2")
                for hi, (i0, n, _, _) in enumerate(HALves):
                    nc.scalar.copy(rtmp[:, i0 : i0 + n], dps[hi][:, :])
                nc.vector.reciprocal_approx_fast(out=rtmp2[:, :], in_=rtmp[:, :])
                nc.scalar.copy(recip_sb[:, :], rtmp2[:, :])
                bps = [psD.tile([128, n], f32, tag="bps", name=f"bps{hi}") for hi, (_, n, _, _) in enumerate(HALves)]
                for hi, (i0, n, _, _) in enumerate(HALves):
                    nc.tensor.matmul(
                        bps[hi][:, :],
                        ones_m[:, :],
                        recip_sb[:, i0 : i0 + n],
                        start=True,
                        stop=True,
                    )
                    nc.scalar.copy(rb_sb[:, i0 : i0 + n], bps[hi][:, :])

            # Reconstruction, one tensor per pass (a then b); pass 0 also
            # applies the softmax denominator to each S.T chunk just before
            # first use so recon matmuls chase the scaling.
            # yaT[c, l'] += sum_{p,q,j} paT[j,(p,q,c)]*S.T[j, i(l',p,q)]
            with ExitStack() as ph2:
                patp = ph2.enter_context(tc.tile_pool(name="pat", bufs=4))
                psY = ph2.enter_context(
                    tc.tile_pool(name="psY", bufs=8, space="PSUM")
                )
                for t, (srcpad, dram) in enumerate(
                    ((a_pad, ya_t), (b_pad, yb_t))
                ):
                    yps = [
                        [
                            psY.tile(
                                [128, n], f32, tag="yps", name=f"yps{t}_{cb}_{hi}"
                            )
                            for hi, (_, n, _, _) in enumerate(HALves)
                        ]
                        for cb in range(2)
                    ]
                    for c, (j0, dm) in enumerate(JC):
                        h0j, nhj = 4 * c, (4 if c < 7 else 3)
                        t3 = tpad[c].rearrange("j (h w) -> j h w", h=PH, w=PW)
                        if t == 0:
                            for hi, (i0, n, h0, nh) in enumerate(HALves):
                                itv = t3[:dm, 1 + h0 : 1 + h0 + nh, 1 : 1 + Wp]
                                nc.vector.tensor_mul(
                                    itv, itv, rb_sb[:dm, i0 : i0 + n]
                                )
                        pt = patp.tile([128, KK], f32r, tag="pat", name=f"pt{t}_{c}")
                        for dh in range(nhj):
                            sap = bass.AP(
                                tensor=srcpad.ap().tensor,
                                offset=(h0j + dh) * PW * C,
                                ap=[
                                    [C, Wp],
                                    [PW * C, 3],
                                    [C, 3],
                                    [1, C],
                                ],
                            )
                            nc.sync.dma_start(
                                out=pt[32 * dh : 32 * (dh + 1), :],
                                in_=sap.bitcast(f32r),
                            )
                        for p in range(3):
                            for q in range(3):
                                for cb in range(2):
                                    lhs = pt[
                                        :dm,
                                        (3 * p + q) * C
                                        + 128 * cb : (3 * p + q) * C
                                        + 128 * (cb + 1),
                                    ]
                                    for hi, (i0, n, h0, nh) in enumerate(HALves):
                                        rhs = t3[
                                            :dm,
                                            h0 - p + 2 : h0 - p + 2 + nh,
                                            2 - q : 2 - q + Wp,
                                        ]
                                        nc.tensor.matmul(
                                            yps[cb][hi][:, :],
                                            lhs,
                                            rhs,
                                            start=(c == 0 and p == 0 and q == 0),
                                            stop=(c == 7 and p == 2 and q == 2),
                                        )

                    for cb in range(2):
                        ysb = outp.tile([128, L], f32, tag="ysb", name=f"ysb{t}_{cb}")
                        for hi, (i0, n, _, _) in enumerate(HALves):
                            nc.vector.tensor_copy(
                                ysb[:, i0 : i0 + n], yps[cb][hi][:, :]
                            )
                        nc.sync.dma_start(
                            out=dram[128 * cb : 128 * (cb + 1), :], in_=ysb[:]
                        )

    nc.compile()
    return nc


def _get_program():
    if "nc" not in _CACHE:
        _CACHE["nc"] = _build_program()
    return _CACHE["nc"]


def _core_inputs(A, B):
    """A, B: [31,32,256] float32 -> per-core input map."""
    ap = np.zeros((PH, PW, C), np.float32)
    ap[1 : 1 + Hp, 1 : 1 + Wp] = A
    bp = np.zeros((PH, PW, C), np.float32)
    bp[1 : 1 + Hp, 1 : 1 + Wp] = B

    def inv_norm(pad):
        s = (pad.astype(np.float64) ** 2).sum(-1)  # [33,34]
        ss = np.zeros((Hp, Wp))
        for p in range(3):
            for q in range(3):
                ss += s[p : p + Hp, q : q + Wp]
        return 1.0 / np.maximum(np.sqrt(ss), 1e-4)

    inv = (inv_norm(ap) * inv_norm(bp)).reshape(-1)  # [992]
    return {
        "a_pad": ap,
        "b_pad": bp,
        "a_chw": np.ascontiguousarray(ap.transpose(2, 0, 1).reshape(C, NPAD)),
        "b_chw": np.ascontiguousarray(bp.transpose(2, 0, 1).reshape(C, NPAD)),
        "inv_p": np.ascontiguousarray(
            np.pad(10.0 * inv, (0, 1024 - L)).reshape(8, 128).T.astype(np.float32)
        ),
        "inv_f": inv.reshape(1, L).astype(np.float32),
    }


def _untp(y_t):
    # [256, 992] channel-major -> [31, 32, 256]
    return y_t.reshape(C, Hp, Wp).transpose(1, 2, 0)


def kernel(x, mask):
    x = np.asarray(x, dtype=np.float32)
    in_maps = []
    for b in range(B_IMG):
        xb = x[b]
        in_maps.append(_core_inputs(xb[:-1], xb[1:]))
        xt = np.ascontiguousarray(xb.transpose(1, 0, 2))
        in_maps.append(_core_inputs(xt[1:], xt[:-1]))

    from concourse.bass_utils import run_bass_kernel_spmd

    nc = _get_program()
    res = run_bass_kernel_spmd(nc, in_maps, list(range(8))).results

    out = np.empty((B_IMG, H_IMG, W_IMG, C), np.float32)
    for b in range(B_IMG):
        yl = _untp(res[2 * b]["ya_t"])
        yr = _untp(res[2 * b]["yb_t"])
        ylr = np.concatenate(
            [yr[:1], (yr[1:] + yl[:-1]) * 0.5, yl[-1:]], axis=0
        )
        yt = _untp(res[2 * b + 1]["ya_t"]).transpose(1, 0, 2)
        yb = _untp(res[2 * b + 1]["yb_t"]).transpose(1, 0, 2)
        ytb = np.concatenate(
            [yt[:, :1], (yt[:, 1:] + yb[:, :-1]) * 0.5, yb[:, -1:]], axis=1
        )
        out[b] = (ylr + ytb) * 0.5
    return out


# revision 13
# speedup vs baseline: 1.0486x; 1.0486x over previous
"""EnvironmentConsistentAttention on 8 trn2 cores.

Sharding: 4 images x 2 directions (vertical/horizontal neighbor pairs) = 8
independent units, one per core. The horizontal direction of image x equals
the vertical direction of x spatially transposed, so a single SPMD program
handles both: given shifted maps A, B [31,32,256] it returns
(yA, yB) = _corr_recon(A, B), each [31,32,256] (emitted channel-major).

Per-core math (Hp=31, Wp=32, C=256, L=992, k=3):
  pa[(p,q,c), l=(h,w)] = A_pad[h+p, w+q, c]          (zero-padded patches)
  z = pa * pb                                        [2304, L]
  R = z.T @ z                                        [L, L] gram
  att[i,j] = inv[i]*inv[j]*R[i,j];  S = softmax(10*att, axis=j)
  yA = conv_transpose(S, pa) -> ya[l',c] = sum_{p,q,j} S[shift(l',p,q), j]*pa[(p,q,c), j]

att is symmetric pre-softmax, so tiles of R computed as [j-part, i-free] are
directly S.T tiles; exp/softmax-denominator (a cross-partition ones-matmul)
and the reconstruction all run in that transposed layout. S.T is stored in a
[33,34]-padded spatial grid over i so the 9 conv-transpose shifts become pure
access-pattern offsets (zero borders give SAME-padding semantics for free).
Patch norms are folded in as row/column scales of R (host precomputes the
tiny [992] inverse-norm vector).
"""

import numpy as np

Hp, Wp, C = 31, 32, 256
L = Hp * Wp            # 992
PH, PW = Hp + 2, Wp + 2  # 33, 34 padded grid
NPAD = PH * PW         # 1122
KK = 9 * C             # 2304
JC = [(128 * c, 128 if c < 7 else 96) for c in range(8)]   # j/l chunks
HALves = [(0, 512, 0, 16), (512, 480, 16, 15)]  # (i0, n, h0, nh) over i/l'
B_IMG, H_IMG, W_IMG = 4, 32, 32

_CACHE = {}


def _build_program():
    import concourse.bass as bass
    import concourse.tile as tile
    from concourse import bacc, mybir

    f32 = mybir.dt.float32
    f32r = mybir.dt.float32r

    def r(ap):
        return ap.bitcast(f32r)

    nc = bacc.Bacc("TRN2", target_bir_lowering=False, debug=False)

    a_pad = nc.dram_tensor("a_pad", [PH, PW, C], f32, kind="ExternalInput")
    b_pad = nc.dram_tensor("b_pad", [PH, PW, C], f32, kind="ExternalInput")
    a_chw = nc.dram_tensor("a_chw", [C, NPAD], f32, kind="ExternalInput")
    b_chw = nc.dram_tensor("b_chw", [C, NPAD], f32, kind="ExternalInput")
    inv_p = nc.dram_tensor("inv_p", [128, 8], f32, kind="ExternalInput")
    inv_f = nc.dram_tensor("inv_f", [1, L], f32, kind="ExternalInput")
    ya_t = nc.dram_tensor("ya_t", [C, L], f32, kind="ExternalOutput")
    yb_t = nc.dram_tensor("yb_t", [C, L], f32, kind="ExternalOutput")

    with tile.TileContext(nc) as tc:
        from contextlib import ExitStack

        with ExitStack() as ctx:
            const = ctx.enter_context(tc.tile_pool(name="const", bufs=1))
            outp = ctx.enter_context(tc.tile_pool(name="outp", bufs=4))
            tpadp = ctx.enter_context(tc.tile_pool(name="tpad", bufs=8))

            # Constants
            sb_inv_p = const.tile([128, 8], f32, tag="invp")
            nc.sync.dma_start(out=sb_inv_p[:], in_=inv_p[:, :])
            sb_inv_b = const.tile([128, L], f32, tag="invb")
            nc.sync.dma_start(
                out=sb_inv_b[:], in_=inv_f.ap().to_broadcast([128, L])
            )
            ones_f = const.tile([128, 128], f32, tag="onesf")
            nc.vector.memset(ones_f[:], 1.0)
            ones_k = const.tile([128, 1], f32r, tag="onesk")
            nc.scalar.copy(ones_k[:], ones_f[:, 0:1])
            ones_m = const.tile([1, 128], f32r, tag="onesm")
            nc.scalar.copy(ones_m[:], ones_f[0:1, :])
            recip_sb = const.tile([1, L], f32r, tag="recip")
            rb_sb = const.tile([128, L], f32, tag="rbcast")

            # S.T tiles in padded-grid layout, zeroed borders
            tpad = []
            for c in range(8):
                t = tpadp.tile([128, NPAD], f32r, tag="tpad")
                tf = t.bitcast(f32).rearrange("j (h w) -> j h w", h=PH, w=PW)
                nc.vector.memset(tf[:, 0:1, :], 0.0)
                nc.vector.memset(tf[:, PH - 1 : PH, :], 0.0)
                nc.vector.memset(tf[:, :, 0:1], 0.0)
                nc.vector.memset(tf[:, :, PW - 1 : PW], 0.0)
                tpad.append(t)

            with ExitStack() as ph1:
                apadp = ph1.enter_context(tc.tile_pool(name="apad", bufs=4))
                zp = ph1.enter_context(tc.tile_pool(name="z", bufs=18))
                psA = ph1.enter_context(
                    tc.tile_pool(name="psA", bufs=4, space="PSUM")
                )
                psD = ph1.enter_context(
                    tc.tile_pool(name="psD", bufs=2, space="PSUM")
                )

                # Load padded inputs channel-major; build z = pa*pb views
                achw, bchw = [], []
                for ch in range(2):
                    ta = apadp.tile([128, NPAD], f32, tag="apad")
                    nc.sync.dma_start(
                        out=ta[:], in_=a_chw[128 * ch : 128 * (ch + 1), :]
                    )
                    achw.append(ta)
                    tb = apadp.tile([128, NPAD], f32, tag="apad")
                    nc.gpsimd.dma_start(
                        out=tb[:], in_=b_chw[128 * ch : 128 * (ch + 1), :]
                    )
                    bchw.append(tb)

                zt = []
                for p in range(3):
                    for q in range(3):
                        for ch in range(2):
                            k = len(zt)
                            zk = zp.tile([128, L], f32r, tag="z")
                            av = achw[ch].rearrange(
                                "c (h w) -> c h w", h=PH, w=PW
                            )[:, p : p + Hp, q : q + Wp]
                            bv = bchw[ch].rearrange(
                                "c (h w) -> c h w", h=PH, w=PW
                            )[:, p : p + Hp, q : q + Wp]
                            nc.vector.tensor_mul(zk[:], av, bv)
                            zt.append(zk)

                # Gram R = z.T@z per (j-chunk, i-half); scale+exp into tpad;
                # accumulate softmax denominators with ones-matmuls.
                dps = [psD.tile([1, n], f32, tag="dps", name=f"dps{hi}") for hi, (_, n, _, _) in enumerate(HALves)]
                for c, (j0, dm) in enumerate(JC):
                    rps = [
                        psA.tile([128, n], f32, tag="rps", name=f"rps{c}_{hi}")
                        for hi, (_, n, _, _) in enumerate(HALves)
                    ]
                    for k in range(18):
                        for hi, (i0, n, _, _) in enumerate(HALves):
                            nc.tensor.matmul(
                                rps[hi][:dm, :],
                                zt[k][:, j0 : j0 + dm],
                                zt[k][:, i0 : i0 + n],
                                start=(k == 0),
                                stop=(k == 17),
                            )
                    t3 = tpad[c].rearrange("j (h w) -> j h w", h=PH, w=PW)
                    for hi, (i0, n, h0, nh) in enumerate(HALves):
                        itv = t3[:dm, 1 + h0 : 1 + h0 + nh, 1 : 1 + Wp]
                        nc.vector.tensor_mul(
                            itv, rps[hi][:dm, :], sb_inv_b[:dm, i0 : i0 + n]
                        )
                        nc.scalar.activation(
                            itv,
                            itv,
                            mybir.ActivationFunctionType.Exp,
                            scale=sb_inv_p[:dm, c : c + 1],
                        )
                        nc.tensor.matmul(
                            dps[hi][:, :],
                            ones_k[:dm, :],
                            t3[:dm, 1 + h0 : 1 + h0 + nh, 1 : 1 + Wp],
                            start=(c == 0),
                            stop=(c == 7),
                        )

                # 1/denom, broadcast across partitions via K=1 matmul
                rtmp = const.tile([1, L], f32, tag="rtmp")
                rtmp2 = const.tile([1, L], f32, tag="rtmp2")
                for hi, (i0, n, _, _) in enumerate(HALves):
                    nc.scalar.copy(rtmp[:, i0 : i0 + n], dps[hi][:, :])
                nc.vector.reciprocal_approx_fast(out=rtmp2[:, :], in_=rtmp[:, :])
                nc.scalar.copy(recip_sb[:, :], rtmp2[:, :])
                bps = [psD.tile([128, n], f32, tag="bps", name=f"bps{hi}") for hi, (_, n, _, _) in enumerate(HALves)]
                for hi, (i0, n, _, _) in enumerate(HALves):
                    nc.tensor.matmul(
                        bps[hi][:, :],
                        ones_m[:, :],
                        recip_sb[:, i0 : i0 + n],
                        start=True,
                        stop=True,
                    )
                    nc.scalar.copy(rb_sb[:, i0 : i0 + n], bps[hi][:, :])

            # Reconstruction, a/b interleaved over one jc sweep; the
            # softmax denominator is applied to each S.T chunk at the top of
            # its jc iteration so recon matmuls chase the scaling.
            # yaT[c, l'] += sum_{p,q,j} paT[j,(p,q,c)]*S.T[j, i(l',p,q)]
            with ExitStack() as ph2:
                patp = ph2.enter_context(tc.tile_pool(name="pat", bufs=4))
                psY = ph2.enter_context(
                    tc.tile_pool(name="psY", bufs=8, space="PSUM")
                )
                yps = [
                    [
                        [
                            psY.tile(
                                [128, n], f32, tag="yps", name=f"yps{t}_{cb}_{hi}"
                            )
                            for hi, (_, n, _, _) in enumerate(HALves)
                        ]
                        for cb in range(2)
                    ]
                    for t in range(2)
                ]
                for c, (j0, dm) in enumerate(JC):
                    h0j, nhj = 4 * c, (4 if c < 7 else 3)
                    t3 = tpad[c].rearrange("j (h w) -> j h w", h=PH, w=PW)
                    for hi, (i0, n, h0, nh) in enumerate(HALves):
                        itv = t3[:dm, 1 + h0 : 1 + h0 + nh, 1 : 1 + Wp]
                        nc.vector.tensor_mul(itv, itv, rb_sb[:dm, i0 : i0 + n])
                    pats = []
                    for t, srcpad in enumerate((a_pad, b_pad)):
                        pt = patp.tile(
                            [128, KK], f32r, tag="pat", name=f"pt{t}_{c}"
                        )
                        for dh in range(nhj):
                            sap = bass.AP(
                                tensor=srcpad.ap().tensor,
                                offset=(h0j + dh) * PW * C,
                                ap=[
                                    [C, Wp],
                                    [PW * C, 3],
                                    [C, 3],
                                    [1, C],
                                ],
                            )
                            eng = nc.sync if t == 0 else nc.gpsimd
                            eng.dma_start(
                                out=pt[32 * dh : 32 * (dh + 1), :],
                                in_=sap.bitcast(f32r),
                            )
                        pats.append(pt)
                    for p in range(3):
                        for q in range(3):
                            for t in range(2):
                                for cb in range(2):
                                    lhs = pats[t][
                                        :dm,
                                        (3 * p + q) * C
                                        + 128 * cb : (3 * p + q) * C
                                        + 128 * (cb + 1),
                                    ]
                                    for hi, (i0, n, h0, nh) in enumerate(HALves):
                                        rhs = t3[
                                            :dm,
                                            h0 - p + 2 : h0 - p + 2 + nh,
                                            2 - q : 2 - q + Wp,
                                        ]
                                        nc.tensor.matmul(
                                            yps[t][cb][hi][:, :],
                                            lhs,
                                            rhs,
                                            start=(c == 0 and p == 0 and q == 0),
                                            stop=(c == 7 and p == 2 and q == 2),
                                        )

                for t, dram in enumerate((ya_t, yb_t)):
                    for cb in range(2):
                        ysb = outp.tile(
                            [128, L], f32, tag="ysb", name=f"ysb{t}_{cb}"
                        )
                        for hi, (i0, n, _, _) in enumerate(HALves):
                            nc.vector.tensor_copy(
                                ysb[:, i0 : i0 + n], yps[t][cb][hi][:, :]
                            )
                        nc.sync.dma_start(
                            out=dram[128 * cb : 128 * (cb + 1), :], in_=ysb[:]
                        )

    nc.compile()
    return nc


def _get_program():
    if "nc" not in _CACHE:
        _CACHE["nc"] = _build_program()
    return _CACHE["nc"]


def _core_inputs(A, B):
    """A, B: [31,32,256] float32 -> per-core input map."""
    ap = np.zeros((PH, PW, C), np.float32)
    ap[1 : 1 + Hp, 1 : 1 + Wp] = A
    bp = np.zeros((PH, PW, C), np.float32)
    bp[1 : 1 + Hp, 1 : 1 + Wp] = B

    def inv_norm(pad):
        s = (pad.astype(np.float64) ** 2).sum(-1)  # [33,34]
        ss = np.zeros((Hp, Wp))
        for p in range(3):
            for q in range(3):
                ss += s[p : p + Hp, q : q + Wp]
        return 1.0 / np.maximum(np.sqrt(ss), 1e-4)

    inv = (inv_norm(ap) * inv_norm(bp)).reshape(-1)  # [992]
    return {
        "a_pad": ap,
        "b_pad": bp,
        "a_chw": np.ascontiguousarray(ap.transpose(2, 0, 1).reshape(C, NPAD)),
        "b_chw": np.ascontiguousarray(bp.transpose(2, 0, 1).reshape(C, NPAD)),
        "inv_p": np.ascontiguousarray(
            np.pad(10.0 * inv, (0, 1024 - L)).reshape(8, 128).T.astype(np.float32)
        ),
        "inv_f": inv.reshape(1, L).astype(np.float32),
    }


def _untp(y_t):
    # [256, 992] channel-major -> [31, 32, 256]
    return y_t.reshape(C, Hp, Wp).transpose(1, 2, 0)


def kernel(x, mask):
    x = np.asarray(x, dtype=np.float32)
    in_maps = []
    for b in range(B_IMG):
        xb = x[b]
        in_maps.append(_core_inputs(xb[:-1], xb[1:]))
        xt = np.ascontiguousarray(xb.transpose(1, 0, 2))
        in_maps.append(_core_inputs(xt[1:], xt[:-1]))

    from concourse.bass_utils import run_bass_kernel_spmd

    nc = _get_program()
    res = run_bass_kernel_spmd(nc, in_maps, list(range(8))).results

    out = np.empty((B_IMG, H_IMG, W_IMG, C), np.float32)
    for b in range(B_IMG):
        yl = _untp(res[2 * b]["ya_t"])
        yr = _untp(res[2 * b]["yb_t"])
        ylr = np.concatenate(
            [yr[:1], (yr[1:] + yl[:-1]) * 0.5, yl[-1:]], axis=0
        )
        yt = _untp(res[2 * b + 1]["ya_t"]).transpose(1, 0, 2)
        yb = _untp(res[2 * b + 1]["yb_t"]).transpose(1, 0, 2)
        ytb = np.concatenate(
            [yt[:, :1], (yt[:, 1:] + yb[:, :-1]) * 0.5, yb[:, -1:]], axis=1
        )
        out[b] = (ylr + ytb) * 0.5
    return out
